# revision 11
# baseline (speedup 1.0000x reference)
"""Trainium2 Bass kernel for nn_Decoder_16183436771335.

Decoder: Bahdanau attention -> tiny CNN -> GRU(503 steps, H=2048) -> MLP head.
dec_units (H=2048) tensor-parallel across 8 NeuronCores; per-step h AllGather.

GRU per-core per-step: psum cols [64r|64z|64nh|64ni] per group g (4 col groups
at partitions 32g, M=32 replicated so gate math runs on 128 partitions).
Contraction: 16 chunks of K=128 (h) + one K=7 chunk ([x;1], biases folded).

Attention trick: V*leaky(a) = 0.6*V*a + 0.4*V*|a|, so the score reduction is
two matvecs over |a| chunks -- no (2048,2048) leaky materialization.
"""
import sys, os

sys.path.insert(0, "/opt/trn_rl_repo")
import numpy as np
import ml_dtypes
from concourse import bass, bacc, tile, mybir, bass_utils

F32 = mybir.dt.float32
F32R = mybir.dt.float32r
BF16 = mybir.dt.bfloat16
AF = mybir.ActivationFunctionType
ALU = mybir.AluOpType
AX = mybir.AxisListType

H = 2048
LP = 503
NCORES = 8
LEAK = 0.2
EPS = 1e-5
NX = 400
NXS = NX // NCORES
L3IN = LP + 4

GRU_BF16 = os.environ.get("GRU_DT", "bf16") == "bf16"
GRU_DT = BF16 if GRU_BF16 else F32


def _leaky(eng, out, in_):
    eng.scalar_tensor_tensor(out, in_, LEAK, in_, ALU.mult, ALU.max)


def _abs(eng, out, in_):
    eng.scalar_tensor_tensor(out, in_, -1.0, in_, ALU.mult, ALU.max)


def build(n_steps=LP, taps=False):
    ns = n_steps
    nc = bacc.Bacc(None, target_bir_lowering=False, num_devices=NCORES)

    def din(name, shape, dt=F32):
        return nc.dram_tensor(name, list(shape), dt, kind="ExternalInput").ap()

    enc7_d = din("enc7", (7, 256))
    w1t7_d = din("w1t7", (7, 2048))
    v1abs_d = din("v1abs", (128, 16))
    v7_d = din("v7", (7, 1))
    w2t_d = din("w2t", (128, 16 * 256))
    b2row_d = din("b2row", (1, 256))
    ones1_d = din("ones1", (1, 1))
    onescol_d = din("onescol", (128, 1))
    onesrow_d = din("onesrow", (1, 128))
    consts_d = din("consts", (1, 16))
    encctx_d = din("encctx", (6, 2048))
    cw_d = {n: din(n + "w", (6, 6 * k)) for n, k in (("c1", 11), ("c2", 11), ("c3", 5), ("c4", 5))}
    cb_d = {n: din(n + "b", (6, 1)) for n in ("c1", "c2", "c3", "c4")}
    bn2g_d = din("bn2g", (6, 1))
    bn2b_d = din("bn2b", (6, 1))
    whrz_d = din("whrz", (128, 16 * 768), GRU_DT)
    wtiny_d = din("wtiny", (7, 1024), GRU_DT)
    h0sb_d = din("h0sb", (128, 16))
    h0loc_d = din("h0loc", (128, 64))
    l1w_d = din("l1w", (128, 16 * NXS))
    l1b_d = din("l1b", (NXS, 1))
    l2w06_d = din("l2w06", (128, 4))
    l2w04_d = din("l2w04", (128, 4))
    l3wt_d = din("l3wt", (128, 4 * 50))
    l3b_d = din("l3b", (1, 50))
    l4wt_d = din("l4wt", (50, 4))
    l4b_d = din("l4b", (1, 4))
    lastin_d = din("lastin", (1, 4))
    ones512_d = din("ones512", (1, 512))
    eye_d = din("eye", (1, 1))

    out_x = nc.dram_tensor("out_x", [1, 4], F32, kind="ExternalOutput").ap()
    out_h = nc.dram_tensor("out_h", [2048], F32, kind="ExternalOutput").ap()
    tap = {}
    if taps:
        for name, shape in (
            ("t_score", [1, 256]), ("t_wt", [128, 16]), ("t_ctx", [6, 2048]),
            ("t_seq", [7, 512]), ("t_hsb", [128, 16]), ("t_hnew", [128, 64]),
            ("t_x50", [NXS, LP]), ("t_u", [1, 512]), ("t_stats", [1, 4]),
            ("t_gate", [128, 256]),
        ):
            tap[name] = nc.dram_tensor(name, shape, F32, kind="ExternalOutput").ap()

    rg = [list(range(NCORES))]

    with tile.TileContext(nc) as tc:
        with (
            tc.tile_pool(name="persist", bufs=1) as P,
            tc.tile_pool(name="work", bufs=2) as W,
            tc.tile_pool(name="dram", bufs=1, space="DRAM") as DR,
            tc.tile_pool(name="dram2", bufs=2, space="DRAM") as DR2,
        ):
            PA_pool = tc.tile_pool(name="phaseA", bufs=1)
            PA = PA_pool.__enter__()

            def load(dap, dt=F32, tag=None, pool=None):
                shape = list(dap.shape)
                t = (pool or P).tile(shape, dt, tag=tag or ("ld_" + dap.tensor.name))
                src = dap[:] if dt == F32 or dt == BF16 else dap[:].bitcast(dt)
                nc.sync.dma_start(out=t[:], in_=src)
                return t

            enc7 = load(enc7_d, F32R, pool=PA)
            w1t7 = load(w1t7_d, F32R, pool=PA)
            v1abs = load(v1abs_d)
            v7 = load(v7_d)
            w2t = load(w2t_d, pool=PA)
            b2row = load(b2row_d)
            ones1 = load(ones1_d)
            onescol = load(onescol_d)
            onesrow = load(onesrow_d)
            consts = load(consts_d)
            encctx = load(encctx_d, pool=PA)
            cw = {k: load(cw_d[k]) for k in cw_d}
            cb = {k: load(cb_d[k]) for k in cb_d}
            bn2g = load(bn2g_d)
            bn2b = load(bn2b_d)
            whrz = load(whrz_d, GRU_DT)
            wtiny = load(wtiny_d, GRU_DT)
            h0sb = load(h0sb_d)
            h0loc = load(h0loc_d)
            l1w = load(l1w_d)
            l1b = load(l1b_d)
            l2w06 = load(l2w06_d)
            l2w04 = load(l2w04_d)
            l3wt = load(l3wt_d)
            l3b = load(l3b_d)
            l4wt = load(l4wt_d)
            l4b = load(l4b_d)
            lastin = load(lastin_d)
            eye1 = load(eye_d)

            # =============== attention score ===============
            att_ps = tc.tile_pool(name="psatt", bufs=1, space="PSUM")
            psA = att_ps.__enter__()
            psS = psA
            # w2pre = W2 @ h0 + b2 (own 256 rows); f32 matmuls
            ps_w2 = psS.tile([1, 256], F32)
            for k in range(16):
                nc.tensor.matmul(ps_w2[:, :], lhsT=h0sb[:, k : k + 1],
                                 rhs=w2t[:, 256 * k : 256 * k + 256],
                                 start=(k == 0), stop=False)
            nc.tensor.matmul(ps_w2[:, :], lhsT=ones1[:, :], rhs=b2row[:, :],
                             start=False, stop=True)
            w2sb = W.tile([1, 256], F32, tag="w2sb")
            nc.vector.tensor_copy(w2sb[:], ps_w2[:])
            w2ab = W.tile([1, 256], F32, tag="w2ab")
            _abs(nc.vector, w2ab[:], w2sb[:])

            ps_sc = psS.tile([1, 256], F32)
            n_mm = 19
            mm = [0]

            def sc_mm(lhsT, rhs):
                nc.tensor.matmul(ps_sc[:, :], lhsT=lhsT, rhs=rhs,
                                 start=(mm[0] == 0), stop=(mm[0] == n_mm - 1))
                mm[0] += 1

            sc_mm(v7[:, :], enc7[:, :].bitcast(F32))
            sc_mm(consts[:, 2:3], w2sb[:])
            sc_mm(consts[:, 3:4], w2ab[:])
            for c in range(16):
                ps_a = psA.tile([128, 256], F32, tag="psbig", bufs=2)
                nc.tensor.matmul(ps_a[:, :], lhsT=w1t7[:, 128 * c : 128 * c + 128],
                                 rhs=enc7[:, :], start=True, stop=True)
                ab = W.tile([128, 256], F32, tag="ab")
                nc.scalar.activation(ab[:], ps_a[:], AF.Abs)
                sc_mm(v1abs[:, c : c + 1], ab[:])
            score = W.tile([1, 256], F32, tag="score")
            nc.vector.tensor_copy(score[:], ps_sc[:])
            if taps:
                nc.sync.dma_start(out=tap["t_score"][:], in_=score[:])

            sc_in = DR.tile([256], F32)
            sc_out = DR.tile([2048], F32)
            nc.sync.dma_start(out=sc_in[:], in_=score[:])
            nc.gpsimd.collective_compute("AllGather", ALU.bypass, replica_groups=rg,
                                         ins=[sc_in.opt()], outs=[sc_out.opt()])
            ssb = P.tile([128, 16], F32)
            nc.sync.dma_start(out=ssb[:], in_=sc_out[:])

            # ---- bn1 + leaky + softmax (replicated) ----
            ssum = W.tile([128, 1], F32, tag="ssum")
            nc.vector.tensor_reduce(ssum[:], ssb[:], AX.X, ALU.add)
            ssqf = W.tile([128, 16], F32, tag="ssqf")
            ssq = W.tile([128, 1], F32, tag="ssq")
            nc.scalar.activation(ssqf[:], ssb[:], AF.Square, accum_out=ssq[:])
            s2 = W.tile([128, 2], F32, tag="s2")
            nc.vector.tensor_copy(s2[:, 0:1], ssum[:])
            nc.vector.tensor_copy(s2[:, 1:2], ssq[:])
            ps_st = psS.tile([1, 2], F32, tag="ps_small", bufs=2)
            nc.tensor.matmul(ps_st[:, :], lhsT=onescol[:, :], rhs=s2[:, :],
                             start=True, stop=True)
            st = W.tile([1, 2], F32, tag="st")
            nc.vector.tensor_scalar(out=st[:], in0=ps_st[:], scalar1=1.0 / H,
                                    scalar2=None, op0=ALU.mult)
            var = W.tile([1, 1], F32, tag="var")
            nc.vector.scalar_tensor_tensor(var[:], st[:, 0:1], -1.0, st[:, 0:1],
                                           ALU.mult, ALU.mult)
            nc.vector.tensor_tensor(out=var[:], in0=st[:, 1:2], in1=var[:], op=ALU.add)
            nc.vector.tensor_scalar(out=var[:], in0=var[:], scalar1=EPS, scalar2=None,
                                    op0=ALU.add)
            nc.scalar.activation(var[:], var[:], AF.Sqrt)
            rinv = W.tile([1, 1], F32, tag="rinv")
            nc.vector.reciprocal(rinv[:], var[:])
            gsc = W.tile([1, 1], F32, tag="gsc")
            nc.vector.tensor_tensor(out=gsc[:], in0=rinv[:], in1=consts[:, 0:1], op=ALU.mult)
            bsc = W.tile([1, 1], F32, tag="bsc")
            nc.vector.tensor_tensor(out=bsc[:], in0=st[:, 0:1], in1=gsc[:], op=ALU.mult)
            nc.vector.scalar_tensor_tensor(bsc[:], bsc[:], -1.0, consts[:, 1:2],
                                           ALU.mult, ALU.add)
            gb2 = W.tile([1, 2], F32, tag="gb2")
            nc.vector.tensor_copy(gb2[:, 0:1], gsc[:])
            nc.vector.tensor_copy(gb2[:, 1:2], bsc[:])
            ps_bc = psS.tile([128, 2], F32, tag="ps_small", bufs=2)
            nc.tensor.matmul(ps_bc[:, :], lhsT=onesrow[:, :], rhs=gb2[:, :],
                             start=True, stop=True)
            gbb = W.tile([128, 2], F32, tag="gbb")
            nc.vector.tensor_copy(gbb[:], ps_bc[:])
            sn = W.tile([128, 16], F32, tag="sn")
            nc.vector.tensor_scalar(out=sn[:], in0=ssb[:], scalar1=gbb[:, 0:1],
                                    scalar2=gbb[:, 1:2], op0=ALU.mult, op1=ALU.add)
            _leaky(nc.vector, sn[:], sn[:])
            esum = W.tile([128, 1], F32, tag="esum")
            ex = W.tile([128, 16], F32, tag="ex")
            nc.scalar.activation(ex[:], sn[:], AF.Exp, accum_out=esum[:])
            ps_z = psS.tile([1, 1], F32, tag="ps_small", bufs=2)
            nc.tensor.matmul(ps_z[:, :], lhsT=onescol[:, :], rhs=esum[:, :],
                             start=True, stop=True)
            zs = W.tile([1, 1], F32, tag="zs")
            nc.vector.tensor_copy(zs[:], ps_z[:])
            zr = W.tile([1, 1], F32, tag="zr")
            nc.vector.reciprocal(zr[:], zs[:])
            ps_zb = psS.tile([128, 1], F32, tag="ps_small", bufs=2)
            nc.tensor.matmul(ps_zb[:, :], lhsT=onesrow[:, :], rhs=zr[:, :],
                             start=True, stop=True)
            zrb = W.tile([128, 1], F32, tag="zrb")
            nc.vector.tensor_copy(zrb[:], ps_zb[:])
            wt = P.tile([128, 16], F32)
            nc.vector.tensor_scalar(out=wt[:], in0=ex[:], scalar1=zrb[:, 0:1],
                                    scalar2=None, op0=ALU.mult)
            if taps:
                nc.sync.dma_start(out=tap["t_wt"][:], in_=wt[:])

            # ---- context ----
            wt_dr = DR.tile([2048], F32)
            nc.sync.dma_start(out=wt_dr[:], in_=wt[:])
            wt6 = PA.tile([6, 2048], F32)
            for ch in range(6):
                nc.sync.dma_start(out=wt6[ch : ch + 1, :], in_=wt_dr[:])
            ctx = PA.tile([6, 2048], F32)
            nc.vector.tensor_tensor(out=ctx[:], in0=encctx[:], in1=wt6[:], op=ALU.mult)
            if taps:
                nc.sync.dma_start(out=tap["t_ctx"][:], in_=ctx[:])

            # =============== CNN ===============
            def conv(src, srclen, wname, kk, pool_after=False):
                L = srclen - kk + 1
                outL = L // 2 if pool_after else L
                dst = PA.tile([6, outL], F32, tag="conv_" + wname)
                nch = (L + 511) // 512
                for j in range(nch):
                    n0 = 512 * j
                    n1 = min(n0 + 512, L)
                    ps_c = psA.tile([6, 512], F32, tag="psbig", bufs=2)
                    for k in range(kk):
                        nc.tensor.matmul(ps_c[:, 0 : n1 - n0],
                                         lhsT=cw[wname][:, 6 * k : 6 * k + 6],
                                         rhs=src[:, n0 + k : n1 + k],
                                         start=(k == 0), stop=(k == kk - 1))
                    seg = W.tile([6, 512], F32, tag="convseg")
                    nc.vector.tensor_scalar(out=seg[:, 0 : n1 - n0],
                                            in0=ps_c[:, 0 : n1 - n0],
                                            scalar1=cb[wname][:, 0:1], scalar2=None,
                                            op0=ALU.add)
                    _leaky(nc.vector, seg[:, 0 : n1 - n0], seg[:, 0 : n1 - n0])
                    if pool_after:
                        nc.vector.tensor_tensor(out=dst[:, n0 // 2 : n1 // 2],
                                                in0=seg[:, 0 : n1 - n0 : 2],
                                                in1=seg[:, 1 : n1 - n0 : 2], op=ALU.max)
                    else:
                        nc.vector.tensor_copy(dst[:, n0:n1], seg[:, 0 : n1 - n0])
                return dst, outL

            c1, L1 = conv(ctx, 2048, "c1", 11)                 # 2038
            c2, L2 = conv(c1, L1, "c2", 11, pool_after=True)   # 1014
            c3, L3 = conv(c2, L2, "c3", 5)                     # 1010
            c4, L4 = conv(c3, L3, "c4", 5)                     # 1006
            csum = W.tile([6, 1], F32, tag="csum")
            nc.vector.tensor_reduce(csum[:], c4[:], AX.X, ALU.add)
            csqf = W.tile([6, 1006], F32, tag="csqf")
            csq = W.tile([6, 1], F32, tag="csq")
            nc.scalar.activation(csqf[:], c4[:], AF.Square, accum_out=csq[:])
            cm = W.tile([6, 1], F32, tag="cm")
            nc.vector.tensor_scalar(out=cm[:], in0=csum[:], scalar1=1.0 / L4,
                                    scalar2=None, op0=ALU.mult)
            cvar = W.tile([6, 1], F32, tag="cvar")
            nc.vector.scalar_tensor_tensor(cvar[:], cm[:], -1.0, cm[:], ALU.mult, ALU.mult)
            nc.vector.tensor_scalar(out=csq[:], in0=csq[:], scalar1=1.0 / L4,
                                    scalar2=None, op0=ALU.mult)
            nc.vector.tensor_tensor(out=cvar[:], in0=csq[:], in1=cvar[:], op=ALU.add)
            nc.vector.tensor_scalar(out=cvar[:], in0=cvar[:], scalar1=EPS, scalar2=None,
                                    op0=ALU.add)
            nc.scalar.activation(cvar[:], cvar[:], AF.Sqrt)
            crinv = W.tile([6, 1], F32, tag="crinv")
            nc.vector.reciprocal(crinv[:], cvar[:])
            cgs = W.tile([6, 1], F32, tag="cgs")
            nc.vector.tensor_tensor(out=cgs[:], in0=crinv[:], in1=bn2g[:], op=ALU.mult)
            cbs = W.tile([6, 1], F32, tag="cbs")
            nc.vector.tensor_tensor(out=cbs[:], in0=cm[:], in1=cgs[:], op=ALU.mult)
            nc.vector.scalar_tensor_tensor(cbs[:], cbs[:], -1.0, bn2b[:], ALU.mult, ALU.add)
            c4n = W.tile([6, 1006], F32, tag="c4n")
            nc.vector.tensor_scalar(out=c4n[:], in0=c4[:], scalar1=cgs[:, 0:1],
                                    scalar2=cbs[:, 0:1], op0=ALU.mult, op1=ALU.add)
            seq7f = P.tile([7, 512], F32)
            nc.vector.memset(seq7f[:], 0.0)
            nc.vector.tensor_tensor(out=seq7f[0:6, 0:503], in0=c4n[:, 0:1006:2],
                                    in1=c4n[:, 1:1006:2], op=ALU.max)
            nc.sync.dma_start(out=seq7f[6:7, :], in_=ones512_d[:])
            if taps:
                nc.sync.dma_start(out=tap["t_seq"][:], in_=seq7f[:])
            seq7 = P.tile([7, 512], GRU_DT)
            nc.vector.tensor_copy(seq7[:], seq7f[:])

            att_ps.__exit__(None, None, None)
            PA_pool.__exit__(None, None, None)

            # =============== GRU ===============
            with (
                tc.tile_pool(name="gsb", bufs=2) as G,
                tc.tile_pool(name="gps", bufs=2, space="PSUM") as GP,
            ):
                hsb = [P.tile([128, 16], F32, name=f"hsb{i}", tag=f"hsb{i}") for i in range(2)]
                nc.vector.tensor_copy(hsb[0][:], h0sb[:])
                hist = P.tile([128, 16 * ns], F32)
                prev_loc = h0loc
                for t in range(ns):
                    cur = hsb[t % 2]
                    nxt = hsb[(t + 1) % 2]
                    hrep = G.tile([128, 512], GRU_DT, tag="hrep")
                    nc.vector.tensor_copy(
                        hrep[:], cur[:].unsqueeze(2).broadcast_to((128, 16, 32)))
                    xrep = G.tile([7, 32], GRU_DT, tag="xrep")
                    nc.vector.tensor_copy(
                        xrep[:], seq7[:, t : t + 1].unsqueeze(2).broadcast_to((7, 1, 32)))
                    ps_g = GP.tile([128, 256], F32, tag="psg")
                    for g in range(4):
                        tp = (0, 32 * g)
                        nc.tensor.matmul(ps_g[32 * g : 32 * g + 32, :],
                                         lhsT=xrep[:, :],
                                         rhs=wtiny[:, 256 * g : 256 * g + 256],
                                         start=True, stop=False, tile_position=tp)
                        for k in range(16):
                            nc.tensor.matmul(
                                ps_g[32 * g : 32 * g + 32, 0:192],
                                lhsT=hrep[:, 32 * k : 32 * k + 32],
                                rhs=whrz[:, 768 * k + 192 * g : 768 * k + 192 * g + 192],
                                start=False, stop=(k == 15), tile_position=tp)
                    rz = G.tile([128, 128], F32, tag="rz")
                    nc.scalar.activation(rz[:], ps_g[:, 0:128], AF.Sigmoid)
                    nt = G.tile([128, 64], F32, tag="nt")
                    nc.vector.tensor_tensor(out=nt[:], in0=rz[:, 0:64],
                                            in1=ps_g[:, 128:192], op=ALU.mult)
                    nc.vector.tensor_tensor(out=nt[:], in0=nt[:],
                                            in1=ps_g[:, 192:256], op=ALU.add)
                    nc.scalar.activation(nt[:], nt[:], AF.Tanh)
                    hnew = G.tile([128, 64], F32, tag="hnew")
                    nc.vector.tensor_tensor(out=hnew[:], in0=prev_loc[:], in1=nt[:],
                                            op=ALU.subtract)
                    nc.vector.tensor_tensor(out=hnew[:], in0=hnew[:], in1=rz[:, 64:128],
                                            op=ALU.mult)
                    nc.vector.tensor_tensor(out=hnew[:], in0=hnew[:], in1=nt[:],
                                            op=ALU.add)
                    prev_loc = hnew
                    h_in = DR2.tile([256], F32, tag="hin")
                    h_out = DR2.tile([2048], F32, tag="hout")
                    nc.sync.dma_start(out=h_in[:], in_=hnew[0:128:32, :])
                    nc.gpsimd.collective_compute("AllGather", ALU.bypass,
                                                 replica_groups=rg,
                                                 ins=[h_in.opt()], outs=[h_out.opt()])
                    nc.sync.dma_start(out=nxt[:], in_=h_out[:])
                    # hist block layout: hist[:, ns*k + t] = leaky(h)[16p+k]
                    _leaky(nc.vector, hist[:, t : t + 15 * ns + 1 : ns], nxt[:])
                    if t == ns - 1:
                        nc.sync.dma_start(out=out_h[:], in_=h_out[:])
                        if taps:
                            nc.sync.dma_start(out=tap["t_hsb"][:], in_=nxt[:])
                            nc.sync.dma_start(out=tap["t_hnew"][:], in_=hnew[:])
                            gf = G.tile([128, 256], F32, tag="gf")
                            nc.vector.tensor_copy(gf[:], ps_g[:])
                            nc.sync.dma_start(out=tap["t_gate"][:], in_=gf[:])

            # =============== MLP head ===============
            head_ps = tc.tile_pool(name="pshead", bufs=1, space="PSUM")
            psS = head_ps.__enter__()
            ps_x = psS.tile([NXS, LP], F32)
            for k in range(16):
                nc.tensor.matmul(ps_x[:, 0:ns],
                                 lhsT=l1w[:, NXS * k : NXS * k + NXS],
                                 rhs=hist[:, ns * k : ns * k + ns],
                                 start=(k == 0), stop=(k == 15))
            x50 = W.tile([NXS, LP], F32, tag="x50")
            nc.vector.tensor_scalar(out=x50[:, 0:ns], in0=ps_x[:, 0:ns],
                                    scalar1=l1b[:, 0:1], scalar2=None, op0=ALU.add)
            if ns < LP:
                nc.vector.memset(x50[:, ns:LP], 0.0)
            if taps:
                nc.sync.dma_start(out=tap["t_x50"][:], in_=x50[:])
            x_in = DR.tile([NXS * LP], F32)
            x_out = DR.tile([NX * LP], F32)
            nc.sync.dma_start(out=x_in[:], in_=x50[:])
            nc.gpsimd.collective_compute("AllGather", ALU.bypass, replica_groups=rg,
                                         ins=[x_in.opt()], outs=[x_out.opt()])
            xch = []
            for c in range(4):
                rows = 128 if c < 3 else NX - 384
                t_ = P.tile([128, LP], F32, tag=f"xch{c}")
                nc.sync.dma_start(out=t_[0:rows, :],
                                  in_=x_out[c * 128 * LP : (c * 128 + rows) * LP])
                xch.append((t_, rows))
            ps_st3 = psS.tile([1, 2], F32, tag="ps_small3", bufs=3)
            for i, (t_, rows) in enumerate(xch):
                xsum = W.tile([128, 1], F32, tag="xsum")
                nc.vector.tensor_reduce(xsum[0:rows, :], t_[0:rows, :], AX.X, ALU.add)
                xsqf = W.tile([128, LP], F32, tag="xsqf")
                xsq = W.tile([128, 1], F32, tag="xsq")
                nc.scalar.activation(xsqf[0:rows, :], t_[0:rows, :], AF.Square,
                                     accum_out=xsq[0:rows, :])
                x2 = W.tile([128, 2], F32, tag="x2")
                if rows < 128:
                    nc.vector.memset(x2[:], 0.0)
                nc.vector.tensor_copy(x2[0:rows, 0:1], xsum[0:rows, :])
                nc.vector.tensor_copy(x2[0:rows, 1:2], xsq[0:rows, :])
                nc.tensor.matmul(ps_st3[:, :], lhsT=onescol[:, :], rhs=x2[:, :],
                                 start=(i == 0), stop=(i == 3))
            NTOT = float(NX * LP)
            st3 = W.tile([1, 2], F32, tag="st3")
            nc.vector.tensor_scalar(out=st3[:], in0=ps_st3[:], scalar1=1.0 / NTOT,
                                    scalar2=None, op0=ALU.mult)
            v3 = W.tile([1, 1], F32, tag="v3")
            nc.vector.scalar_tensor_tensor(v3[:], st3[:, 0:1], -1.0, st3[:, 0:1],
                                           ALU.mult, ALU.mult)
            nc.vector.tensor_tensor(out=v3[:], in0=st3[:, 1:2], in1=v3[:], op=ALU.add)
            nc.vector.tensor_scalar(out=v3[:], in0=v3[:], scalar1=EPS, scalar2=None,
                                    op0=ALU.add)
            nc.scalar.activation(v3[:], v3[:], AF.Sqrt)
            r3 = W.tile([1, 1], F32, tag="r3")
            nc.vector.reciprocal(r3[:], v3[:])
            g3 = W.tile([1, 1], F32, tag="g3")
            nc.vector.tensor_tensor(out=g3[:], in0=r3[:], in1=consts[:, 4:5], op=ALU.mult)
            b3 = W.tile([1, 1], F32, tag="b3")
            nc.vector.tensor_tensor(out=b3[:], in0=st3[:, 0:1], in1=g3[:], op=ALU.mult)
            nc.vector.scalar_tensor_tensor(b3[:], b3[:], -1.0, consts[:, 5:6],
                                           ALU.mult, ALU.add)
            if taps:
                stt_ = W.tile([1, 4], F32, tag="stt_")
                nc.vector.tensor_copy(stt_[:, 0:2], st3[:])
                nc.vector.tensor_copy(stt_[:, 2:3], g3[:])
                nc.vector.tensor_copy(stt_[:, 3:4], b3[:])
                nc.sync.dma_start(out=tap["t_stats"][:], in_=stt_[:])
            gb3 = W.tile([1, 2], F32, tag="gb3")
            nc.vector.tensor_copy(gb3[:, 0:1], g3[:])
            nc.vector.tensor_copy(gb3[:, 1:2], b3[:])
            ps_bc3 = psS.tile([128, 2], F32, tag="ps_small3", bufs=3)
            nc.tensor.matmul(ps_bc3[:, :], lhsT=onesrow[:, :], rhs=gb3[:, :],
                             start=True, stop=True)
            g3b = W.tile([128, 2], F32, tag="g3b")
            nc.vector.tensor_copy(g3b[:], ps_bc3[:])
            ps_y = psS.tile([1, LP], F32, tag="ps_small3", bufs=3)
            for i, (t_, rows) in enumerate(xch):
                v_ = W.tile([128, LP], F32, tag="v_")
                va = W.tile([128, LP], F32, tag="va")
                if rows < 128:
                    nc.vector.memset(v_[:], 0.0)
                    nc.vector.memset(va[:], 0.0)
                nc.vector.tensor_scalar(out=v_[0:rows, :], in0=t_[0:rows, :],
                                        scalar1=g3b[0:rows, 0:1], scalar2=g3b[0:rows, 1:2],
                                        op0=ALU.mult, op1=ALU.add)
                _abs(nc.vector, va[0:rows, :], v_[0:rows, :])
                nc.tensor.matmul(ps_y[:, :], lhsT=l2w06[:, i : i + 1], rhs=v_[:, :],
                                 start=(i == 0), stop=False)
                nc.tensor.matmul(ps_y[:, :], lhsT=l2w04[:, i : i + 1], rhs=va[:, :],
                                 start=False, stop=(i == 3))
            u_sb = P.tile([1, 512], F32)
            nc.vector.memset(u_sb[:], 0.0)
            nc.vector.tensor_scalar(out=u_sb[:, 0:LP], in0=ps_y[:, :],
                                    scalar1=consts[:, 6:7], scalar2=None, op0=ALU.add)
            _leaky(nc.vector, u_sb[:, 0:LP], u_sb[:, 0:LP])
            nc.vector.tensor_copy(u_sb[:, LP : LP + 4], lastin[:])
            if taps:
                nc.sync.dma_start(out=tap["t_u"][:], in_=u_sb[:])
            ps_u = psS.tile([128, 4], F32, tag="ps_small3", bufs=3)
            for c in range(4):
                nc.tensor.transpose(ps_u[:, c : c + 1],
                                    u_sb[:, 128 * c : 128 * c + 128], eye1[:, :])
            ut = W.tile([128, 4], F32, tag="ut")
            nc.vector.tensor_copy(ut[:], ps_u[:])
            ps_v = psS.tile([1, 50], F32, tag="ps_small3", bufs=3)
            for c in range(4):
                nc.tensor.matmul(ps_v[:, :], lhsT=ut[:, c : c + 1],
                                 rhs=l3wt[:, 50 * c : 50 * c + 50],
                                 start=(c == 0), stop=(c == 3))
            vv = W.tile([1, 50], F32, tag="vv")
            nc.vector.tensor_tensor(out=vv[:], in0=ps_v[:], in1=l3b[:], op=ALU.add)
            _leaky(nc.vector, vv[:], vv[:])
            ps_vt = psS.tile([50, 1], F32, tag="ps_small3", bufs=3)
            nc.tensor.transpose(ps_vt[:, :], vv[:, :], eye1[:, :])
            vt = W.tile([50, 1], F32, tag="vt")
            nc.vector.tensor_copy(vt[:], ps_vt[:])
            ps_o = psS.tile([1, 4], F32, tag="ps_small3", bufs=3)
            nc.tensor.matmul(ps_o[:, :], lhsT=vt[:, :], rhs=l4wt[:, :],
                             start=True, stop=True)
            xout = W.tile([1, 4], F32, tag="xout")
            nc.vector.tensor_tensor(out=xout[:], in0=ps_o[:], in1=l4b[:], op=ALU.add)
            nc.sync.dma_start(out=out_x[:], in_=xout[:])
            head_ps.__exit__(None, None, None)

    nc.compile()
    return nc


# ==================== host-side prep ====================

def prep_inputs(inp):
    f = lambda x: np.ascontiguousarray(np.asarray(x, dtype=np.float32))
    W1, b1 = f(inp["W1_w"]), f(inp["W1_b"])
    V = f(inp["V_w"])[0]
    Vb = float(np.asarray(inp["V_b"]).reshape(-1)[0])
    vH = float(V[H])
    Vh = V[:H]
    enc = f(inp["enc_output"])[:, 0, :]
    hidden = f(inp["hidden"]).reshape(-1)
    W2, b2 = f(inp["W2_w"]), f(inp["W2_b"])
    Wi, Whh = f(inp["gru_Wi"]), f(inp["gru_Wh"])
    bi, bh = f(inp["gru_bi"]), f(inp["gru_bh"])
    lin1w, lin1b = f(inp["lin1_w"]), f(inp["lin1_b"])
    lin2w = f(inp["lin2_w"])[0]
    lin2b = float(np.asarray(inp["lin2_b"]).reshape(-1)[0])
    lin3w, lin3b = f(inp["lin3_w"]), f(inp["lin3_b"])
    lin4w, lin4b = f(inp["lin4_w"]), f(inp["lin4_b"])

    vW1 = 0.6 * (Vh @ W1)
    vc = 0.6 * float(Vh @ b1) + Vb
    v7 = np.concatenate([vW1, [vc]]).astype(np.float32).reshape(7, 1)
    w1t7 = np.concatenate([W1.T, b1[None, :]], axis=0).astype(np.float32)
    v1abs_t = (0.4 * Vh).reshape(16, 128).T.copy()

    consts = np.zeros((1, 16), np.float32)
    consts[0, 0] = float(np.asarray(inp["bn1_g"]).reshape(-1)[0])
    consts[0, 1] = float(np.asarray(inp["bn1_b"]).reshape(-1)[0])
    consts[0, 2] = 0.6 * vH
    consts[0, 3] = 0.4 * vH
    consts[0, 4] = float(np.asarray(inp["bn3_g"]).reshape(-1)[0])
    consts[0, 5] = float(np.asarray(inp["bn3_b"]).reshape(-1)[0])
    consts[0, 6] = lin2b

    def convw(w):
        kk = w.shape[2]
        out = np.zeros((6, 6 * kk), np.float32)
        for k in range(kk):
            out[:, 6 * k : 6 * k + 6] = w[:, :, k].T
        return out

    Wh_r, Wh_z, Wh_n = Whh[:H], Whh[H : 2 * H], Whh[2 * H :]
    Wi_r, Wi_z, Wi_n = Wi[:H], Wi[H : 2 * H], Wi[2 * H :]
    bi_r, bi_z, bi_n = bi[:H], bi[H : 2 * H], bi[2 * H :]
    bh_r, bh_z, bh_n = bh[:H], bh[H : 2 * H], bh[2 * H :]

    gdt = ml_dtypes.bfloat16 if GRU_BF16 else np.float32
    in_maps = []
    for c in range(NCORES):
        sl = slice(256 * c, 256 * c + 256)
        w2t = np.zeros((128, 16 * 256), np.float32)
        for k in range(16):
            w2t[:, 256 * k : 256 * k + 256] = W2[sl][:, k::16].T
        whrz = np.zeros((128, 16 * 768), np.float32)
        for k in range(16):
            for g in range(4):
                usl = slice(256 * c + 64 * g, 256 * c + 64 * g + 64)
                b0 = 768 * k + 192 * g
                whrz[:, b0 : b0 + 64] = Wh_r[usl][:, k::16].T
                whrz[:, b0 + 64 : b0 + 128] = Wh_z[usl][:, k::16].T
                whrz[:, b0 + 128 : b0 + 192] = Wh_n[usl][:, k::16].T
        wtiny = np.zeros((7, 1024), np.float32)
        for g in range(4):
            usl = slice(256 * c + 64 * g, 256 * c + 64 * g + 64)
            b0 = 256 * g
            wtiny[0:6, b0 : b0 + 64] = Wi_r[usl].T
            wtiny[6, b0 : b0 + 64] = bi_r[usl] + bh_r[usl]
            wtiny[0:6, b0 + 64 : b0 + 128] = Wi_z[usl].T
            wtiny[6, b0 + 64 : b0 + 128] = bi_z[usl] + bh_z[usl]
            wtiny[6, b0 + 128 : b0 + 192] = bh_n[usl]
            wtiny[0:6, b0 + 192 : b0 + 256] = Wi_n[usl].T
            wtiny[6, b0 + 192 : b0 + 256] = bi_n[usl]
        l1w = np.zeros((128, 16 * NXS), np.float32)
        rsl = slice(NXS * c, NXS * c + NXS)
        for k in range(16):
            l1w[:, NXS * k : NXS * k + NXS] = lin1w[rsl][:, k::16].T
        l2w06 = np.zeros((128, 4), np.float32)
        l2w04 = np.zeros((128, 4), np.float32)
        for cc in range(4):
            rows = 128 if cc < 3 else NX - 384
            l2w06[0:rows, cc] = 0.6 * lin2w[128 * cc : 128 * cc + rows]
            l2w04[0:rows, cc] = 0.4 * lin2w[128 * cc : 128 * cc + rows]
        l3wt = np.zeros((128, 4 * 50), np.float32)
        for cc in range(4):
            nn = 128 if cc < 3 else L3IN - 384
            l3wt[0:nn, 50 * cc : 50 * cc + 50] = lin3w[:, 128 * cc : 128 * cc + nn].T
        m = {
            "enc7": np.concatenate(
                [enc[sl].T, np.ones((1, 256), np.float32)], axis=0),
            "w1t7": w1t7, "v1abs": v1abs_t, "v7": v7,
            "w2t": w2t, "b2row": b2[sl].reshape(1, 256),
            "ones1": np.ones((1, 1), np.float32),
            "onescol": np.ones((128, 1), np.float32),
            "onesrow": np.ones((1, 128), np.float32),
            "consts": consts, "encctx": enc.T,
            "c1w": convw(f(inp["c1_w"])), "c1b": f(inp["c1_b"]).reshape(6, 1),
            "c2w": convw(f(inp["c2_w"])), "c2b": f(inp["c2_b"]).reshape(6, 1),
            "c3w": convw(f(inp["c3_w"])), "c3b": f(inp["c3_b"]).reshape(6, 1),
            "c4w": convw(f(inp["c4_w"])), "c4b": f(inp["c4_b"]).reshape(6, 1),
            "bn2g": f(inp["bn2_g"]).reshape(6, 1), "bn2b": f(inp["bn2_b"]).reshape(6, 1),
            "whrz": whrz.astype(gdt), "wtiny": wtiny.astype(gdt),
            "h0sb": hidden.reshape(128, 16),
            "h0loc": np.repeat(hidden[sl].reshape(4, 64), 32, axis=0),
            "l1w": l1w, "l1b": lin1b[rsl].reshape(NXS, 1),
            "l2w06": l2w06, "l2w04": l2w04,
            "l3wt": l3wt, "l3b": lin3b.reshape(1, 50),
            "l4wt": lin4w.T, "l4b": lin4b.reshape(1, 4),
            "lastin": f(inp["last_input"]).reshape(1, 4),
            "ones512": np.ones((1, 512), np.float32),
            "eye": np.ones((1, 1), np.float32),
        }
        in_maps.append({k: np.ascontiguousarray(v) for k, v in m.items()})
    return in_maps


_CACHE = {}


def kernel(**inputs):
    if "nc" not in _CACHE:
        _CACHE["nc"] = build(n_steps=LP, taps=False)
    in_maps = prep_inputs(inputs)
    res = bass_utils.run_bass_kernel_spmd(_CACHE["nc"], in_maps,
                                          core_ids=list(range(NCORES)))
    x = res.results[0]["out_x"].reshape(4).astype(np.float32)
    hT = res.results[0]["out_h"].reshape(1, 1, 2048).astype(np.float32)
    return x, hT


# revision 13
# speedup vs baseline: 1.1710x; 1.1710x over previous
"""Trainium2 Bass kernel for nn_Decoder_16183436771335.

Decoder: Bahdanau attention -> tiny CNN -> GRU(503 steps, H=2048) -> MLP head.
dec_units (H=2048) tensor-parallel across 8 NeuronCores; per-step h AllGather.

GRU per-core per-step: psum cols [64r|64z|64nh|64ni] per group g (4 col groups
at partitions 32g, M=32 replicated so gate math runs on 128 partitions).
Contraction: 16 chunks of K=128 (h) + one K=7 chunk ([x;1], biases folded).

Attention trick: V*leaky(a) = 0.6*V*a + 0.4*V*|a|, so the score reduction is
two matvecs over |a| chunks -- no (2048,2048) leaky materialization.
"""
import sys, os

sys.path.insert(0, "/opt/trn_rl_repo")
import numpy as np
import ml_dtypes
from concourse import bass, bacc, tile, mybir, bass_utils

F32 = mybir.dt.float32
F32R = mybir.dt.float32r
BF16 = mybir.dt.bfloat16
AF = mybir.ActivationFunctionType
ALU = mybir.AluOpType
AX = mybir.AxisListType

H = 2048
LP = 503
NCORES = 8
LEAK = 0.2
EPS = 1e-5
NX = 400
NXS = NX // NCORES
L3IN = LP + 4

GRU_BF16 = os.environ.get("GRU_DT", "bf16") == "bf16"
GRU_DT = BF16 if GRU_BF16 else F32


def _leaky(eng, out, in_):
    eng.scalar_tensor_tensor(out, in_, LEAK, in_, ALU.mult, ALU.max)


def _abs(eng, out, in_):
    eng.scalar_tensor_tensor(out, in_, -1.0, in_, ALU.mult, ALU.max)


def build(n_steps=LP, taps=False):
    ns = n_steps
    nc = bacc.Bacc(None, target_bir_lowering=False, num_devices=NCORES)

    def din(name, shape, dt=F32):
        return nc.dram_tensor(name, list(shape), dt, kind="ExternalInput").ap()

    enc7_d = din("enc7", (7, 256))
    w1t7_d = din("w1t7", (7, 2048))
    v1abs_d = din("v1abs", (128, 16))
    v7_d = din("v7", (7, 1))
    w2t_d = din("w2t", (128, 16 * 256))
    b2row_d = din("b2row", (1, 256))
    ones1_d = din("ones1", (1, 1))
    onescol_d = din("onescol", (128, 1))
    onesrow_d = din("onesrow", (1, 128))
    consts_d = din("consts", (1, 16))
    encctx_d = din("encctx", (6, 2048))
    cw_d = {n: din(n + "w", (6, 6 * k)) for n, k in (("c1", 11), ("c2", 11), ("c3", 5), ("c4", 5))}
    cb_d = {n: din(n + "b", (6, 1)) for n in ("c1", "c2", "c3", "c4")}
    bn2g_d = din("bn2g", (6, 1))
    bn2b_d = din("bn2b", (6, 1))
    whrz_d = din("whrz", (128, 16 * 768), GRU_DT)
    wtiny_d = din("wtiny", (7, 1024), GRU_DT)
    h0sb_d = din("h0sb", (128, 16))
    h0rep_d = din("h0rep", (128, 512), GRU_DT)
    h0loc_d = din("h0loc", (128, 64))
    l1w_d = din("l1w", (128, 16 * NXS), GRU_DT)
    l1b_d = din("l1b", (NXS, 1))
    l2w06_d = din("l2w06", (128, 4))
    l2w04_d = din("l2w04", (128, 4))
    l3wt_d = din("l3wt", (128, 4 * 50))
    l3b_d = din("l3b", (1, 50))
    l4wt_d = din("l4wt", (50, 4))
    l4b_d = din("l4b", (1, 4))
    lastin_d = din("lastin", (1, 4))
    ones512_d = din("ones512", (1, 512))
    eye_d = din("eye", (1, 1))

    out_x = nc.dram_tensor("out_x", [1, 4], F32, kind="ExternalOutput").ap()
    out_h = nc.dram_tensor("out_h", [2048], F32, kind="ExternalOutput").ap()
    tap = {}
    if taps:
        for name, shape in (
            ("t_score", [1, 256]), ("t_wt", [128, 16]), ("t_ctx", [6, 2048]),
            ("t_seq", [7, 512]), ("t_hsb", [128, 16]), ("t_hnew", [128, 64]),
            ("t_x50", [NXS, LP]), ("t_u", [1, 512]), ("t_stats", [1, 4]),
            ("t_gate", [128, 256]),
        ):
            tap[name] = nc.dram_tensor(name, shape, F32, kind="ExternalOutput").ap()

    rg = [list(range(NCORES))]

    with tile.TileContext(nc) as tc:
        with (
            tc.tile_pool(name="persist", bufs=1) as P,
            tc.tile_pool(name="work", bufs=2) as W,
            tc.tile_pool(name="dram", bufs=1, space="DRAM") as DR,
            tc.tile_pool(name="dram2", bufs=2, space="DRAM") as DR2,
        ):
            PA_pool = tc.tile_pool(name="phaseA", bufs=1)
            PA = PA_pool.__enter__()

            def load(dap, dt=F32, tag=None, pool=None):
                shape = list(dap.shape)
                t = (pool or P).tile(shape, dt, tag=tag or ("ld_" + dap.tensor.name))
                src = dap[:] if dt == F32 or dt == BF16 else dap[:].bitcast(dt)
                nc.sync.dma_start(out=t[:], in_=src)
                return t

            enc7 = load(enc7_d, F32R, pool=PA)
            w1t7 = load(w1t7_d, F32R, pool=PA)
            v1abs = load(v1abs_d)
            v7 = load(v7_d)
            w2t = load(w2t_d, pool=PA)
            b2row = load(b2row_d)
            ones1 = load(ones1_d)
            onescol = load(onescol_d)
            onesrow = load(onesrow_d)
            consts = load(consts_d)
            encctx = load(encctx_d, pool=PA)
            cw = {k: load(cw_d[k]) for k in cw_d}
            cb = {k: load(cb_d[k]) for k in cb_d}
            bn2g = load(bn2g_d)
            bn2b = load(bn2b_d)
            whrz = load(whrz_d, GRU_DT)
            wtiny = load(wtiny_d, GRU_DT)
            h0sb = load(h0sb_d)
            h0loc = load(h0loc_d)
            l1w = load(l1w_d, GRU_DT)
            l1b = load(l1b_d)
            l2w06 = load(l2w06_d)
            l2w04 = load(l2w04_d)
            l3wt = load(l3wt_d)
            l3b = load(l3b_d)
            l4wt = load(l4wt_d)
            l4b = load(l4b_d)
            lastin = load(lastin_d)
            eye1 = load(eye_d)

            # =============== attention score ===============
            att_ps = tc.tile_pool(name="psatt", bufs=1, space="PSUM")
            psA = att_ps.__enter__()
            psS = psA
            # w2pre = W2 @ h0 + b2 (own 256 rows); f32 matmuls
            ps_w2 = psS.tile([1, 256], F32)
            for k in range(16):
                nc.tensor.matmul(ps_w2[:, :], lhsT=h0sb[:, k : k + 1],
                                 rhs=w2t[:, 256 * k : 256 * k + 256],
                                 start=(k == 0), stop=False)
            nc.tensor.matmul(ps_w2[:, :], lhsT=ones1[:, :], rhs=b2row[:, :],
                             start=False, stop=True)
            w2sb = W.tile([1, 256], F32, tag="w2sb")
            nc.vector.tensor_copy(w2sb[:], ps_w2[:])
            w2ab = W.tile([1, 256], F32, tag="w2ab")
            _abs(nc.vector, w2ab[:], w2sb[:])

            ps_sc = psS.tile([1, 256], F32)
            n_mm = 19
            mm = [0]

            def sc_mm(lhsT, rhs):
                nc.tensor.matmul(ps_sc[:, :], lhsT=lhsT, rhs=rhs,
                                 start=(mm[0] == 0), stop=(mm[0] == n_mm - 1))
                mm[0] += 1

            sc_mm(v7[:, :], enc7[:, :].bitcast(F32))
            sc_mm(consts[:, 2:3], w2sb[:])
            sc_mm(consts[:, 3:4], w2ab[:])
            for c in range(16):
                ps_a = psA.tile([128, 256], F32, tag="psbig", bufs=2)
                nc.tensor.matmul(ps_a[:, :], lhsT=w1t7[:, 128 * c : 128 * c + 128],
                                 rhs=enc7[:, :], start=True, stop=True)
                ab = W.tile([128, 256], F32, tag="ab")
                nc.scalar.activation(ab[:], ps_a[:], AF.Abs)
                sc_mm(v1abs[:, c : c + 1], ab[:])
            score = W.tile([1, 256], F32, tag="score")
            nc.vector.tensor_copy(score[:], ps_sc[:])
            if taps:
                nc.sync.dma_start(out=tap["t_score"][:], in_=score[:])

            sc_in = DR.tile([256], F32)
            sc_out = DR.tile([2048], F32)
            nc.sync.dma_start(out=sc_in[:], in_=score[:])
            nc.gpsimd.collective_compute("AllGather", ALU.bypass, replica_groups=rg,
                                         ins=[sc_in.opt()], outs=[sc_out.opt()])
            ssb = P.tile([128, 16], F32)
            nc.sync.dma_start(out=ssb[:], in_=sc_out[:])

            # ---- bn1 + leaky + softmax (replicated) ----
            ssum = W.tile([128, 1], F32, tag="ssum")
            nc.vector.tensor_reduce(ssum[:], ssb[:], AX.X, ALU.add)
            ssqf = W.tile([128, 16], F32, tag="ssqf")
            ssq = W.tile([128, 1], F32, tag="ssq")
            nc.scalar.activation(ssqf[:], ssb[:], AF.Square, accum_out=ssq[:])
            s2 = W.tile([128, 2], F32, tag="s2")
            nc.vector.tensor_copy(s2[:, 0:1], ssum[:])
            nc.vector.tensor_copy(s2[:, 1:2], ssq[:])
            ps_st = psS.tile([1, 2], F32, tag="ps_small", bufs=2)
            nc.tensor.matmul(ps_st[:, :], lhsT=onescol[:, :], rhs=s2[:, :],
                             start=True, stop=True)
            st = W.tile([1, 2], F32, tag="st")
            nc.vector.tensor_scalar(out=st[:], in0=ps_st[:], scalar1=1.0 / H,
                                    scalar2=None, op0=ALU.mult)
            var = W.tile([1, 1], F32, tag="var")
            nc.vector.scalar_tensor_tensor(var[:], st[:, 0:1], -1.0, st[:, 0:1],
                                           ALU.mult, ALU.mult)
            nc.vector.tensor_tensor(out=var[:], in0=st[:, 1:2], in1=var[:], op=ALU.add)
            nc.vector.tensor_scalar(out=var[:], in0=var[:], scalar1=EPS, scalar2=None,
                                    op0=ALU.add)
            nc.scalar.activation(var[:], var[:], AF.Sqrt)
            rinv = W.tile([1, 1], F32, tag="rinv")
            nc.vector.reciprocal(rinv[:], var[:])
            gsc = W.tile([1, 1], F32, tag="gsc")
            nc.vector.tensor_tensor(out=gsc[:], in0=rinv[:], in1=consts[:, 0:1], op=ALU.mult)
            bsc = W.tile([1, 1], F32, tag="bsc")
            nc.vector.tensor_tensor(out=bsc[:], in0=st[:, 0:1], in1=gsc[:], op=ALU.mult)
            nc.vector.scalar_tensor_tensor(bsc[:], bsc[:], -1.0, consts[:, 1:2],
                                           ALU.mult, ALU.add)
            gb2 = W.tile([1, 2], F32, tag="gb2")
            nc.vector.tensor_copy(gb2[:, 0:1], gsc[:])
            nc.vector.tensor_copy(gb2[:, 1:2], bsc[:])
            ps_bc = psS.tile([128, 2], F32, tag="ps_small", bufs=2)
            nc.tensor.matmul(ps_bc[:, :], lhsT=onesrow[:, :], rhs=gb2[:, :],
                             start=True, stop=True)
            gbb = W.tile([128, 2], F32, tag="gbb")
            nc.vector.tensor_copy(gbb[:], ps_bc[:])
            sn = W.tile([128, 16], F32, tag="sn")
            nc.vector.tensor_scalar(out=sn[:], in0=ssb[:], scalar1=gbb[:, 0:1],
                                    scalar2=gbb[:, 1:2], op0=ALU.mult, op1=ALU.add)
            _leaky(nc.vector, sn[:], sn[:])
            esum = W.tile([128, 1], F32, tag="esum")
            ex = W.tile([128, 16], F32, tag="ex")
            nc.scalar.activation(ex[:], sn[:], AF.Exp, accum_out=esum[:])
            ps_z = psS.tile([1, 1], F32, tag="ps_small", bufs=2)
            nc.tensor.matmul(ps_z[:, :], lhsT=onescol[:, :], rhs=esum[:, :],
                             start=True, stop=True)
            zs = W.tile([1, 1], F32, tag="zs")
            nc.vector.tensor_copy(zs[:], ps_z[:])
            zr = W.tile([1, 1], F32, tag="zr")
            nc.vector.reciprocal(zr[:], zs[:])
            ps_zb = psS.tile([128, 1], F32, tag="ps_small", bufs=2)
            nc.tensor.matmul(ps_zb[:, :], lhsT=onesrow[:, :], rhs=zr[:, :],
                             start=True, stop=True)
            zrb = W.tile([128, 1], F32, tag="zrb")
            nc.vector.tensor_copy(zrb[:], ps_zb[:])
            wt = P.tile([128, 16], F32)
            nc.vector.tensor_scalar(out=wt[:], in0=ex[:], scalar1=zrb[:, 0:1],
                                    scalar2=None, op0=ALU.mult)
            if taps:
                nc.sync.dma_start(out=tap["t_wt"][:], in_=wt[:])

            # ---- context ----
            wt_dr = DR.tile([2048], F32)
            nc.sync.dma_start(out=wt_dr[:], in_=wt[:])
            wt6 = PA.tile([6, 2048], F32)
            for ch in range(6):
                nc.sync.dma_start(out=wt6[ch : ch + 1, :], in_=wt_dr[:])
            ctx = PA.tile([6, 2048], F32)
            nc.vector.tensor_tensor(out=ctx[:], in0=encctx[:], in1=wt6[:], op=ALU.mult)
            if taps:
                nc.sync.dma_start(out=tap["t_ctx"][:], in_=ctx[:])

            # =============== CNN ===============
            def conv(src, srclen, wname, kk, pool_after=False):
                L = srclen - kk + 1
                outL = L // 2 if pool_after else L
                dst = PA.tile([6, outL], F32, tag="conv_" + wname)
                nch = (L + 511) // 512
                for j in range(nch):
                    n0 = 512 * j
                    n1 = min(n0 + 512, L)
                    ps_c = psA.tile([6, 512], F32, tag="psbig", bufs=2)
                    for k in range(kk):
                        nc.tensor.matmul(ps_c[:, 0 : n1 - n0],
                                         lhsT=cw[wname][:, 6 * k : 6 * k + 6],
                                         rhs=src[:, n0 + k : n1 + k],
                                         start=(k == 0), stop=(k == kk - 1))
                    seg = W.tile([6, 512], F32, tag="convseg")
                    nc.vector.tensor_scalar(out=seg[:, 0 : n1 - n0],
                                            in0=ps_c[:, 0 : n1 - n0],
                                            scalar1=cb[wname][:, 0:1], scalar2=None,
                                            op0=ALU.add)
                    _leaky(nc.vector, seg[:, 0 : n1 - n0], seg[:, 0 : n1 - n0])
                    if pool_after:
                        nc.vector.tensor_tensor(out=dst[:, n0 // 2 : n1 // 2],
                                                in0=seg[:, 0 : n1 - n0 : 2],
                                                in1=seg[:, 1 : n1 - n0 : 2], op=ALU.max)
                    else:
                        nc.vector.tensor_copy(dst[:, n0:n1], seg[:, 0 : n1 - n0])
                return dst, outL

            c1, L1 = conv(ctx, 2048, "c1", 11)                 # 2038
            c2, L2 = conv(c1, L1, "c2", 11, pool_after=True)   # 1014
            c3, L3 = conv(c2, L2, "c3", 5)                     # 1010
            c4, L4 = conv(c3, L3, "c4", 5)                     # 1006
            csum = W.tile([6, 1], F32, tag="csum")
            nc.vector.tensor_reduce(csum[:], c4[:], AX.X, ALU.add)
            csqf = W.tile([6, 1006], F32, tag="csqf")
            csq = W.tile([6, 1], F32, tag="csq")
            nc.scalar.activation(csqf[:], c4[:], AF.Square, accum_out=csq[:])
            cm = W.tile([6, 1], F32, tag="cm")
            nc.vector.tensor_scalar(out=cm[:], in0=csum[:], scalar1=1.0 / L4,
                                    scalar2=None, op0=ALU.mult)
            cvar = W.tile([6, 1], F32, tag="cvar")
            nc.vector.scalar_tensor_tensor(cvar[:], cm[:], -1.0, cm[:], ALU.mult, ALU.mult)
            nc.vector.tensor_scalar(out=csq[:], in0=csq[:], scalar1=1.0 / L4,
                                    scalar2=None, op0=ALU.mult)
            nc.vector.tensor_tensor(out=cvar[:], in0=csq[:], in1=cvar[:], op=ALU.add)
            nc.vector.tensor_scalar(out=cvar[:], in0=cvar[:], scalar1=EPS, scalar2=None,
                                    op0=ALU.add)
            nc.scalar.activation(cvar[:], cvar[:], AF.Sqrt)
            crinv = W.tile([6, 1], F32, tag="crinv")
            nc.vector.reciprocal(crinv[:], cvar[:])
            cgs = W.tile([6, 1], F32, tag="cgs")
            nc.vector.tensor_tensor(out=cgs[:], in0=crinv[:], in1=bn2g[:], op=ALU.mult)
            cbs = W.tile([6, 1], F32, tag="cbs")
            nc.vector.tensor_tensor(out=cbs[:], in0=cm[:], in1=cgs[:], op=ALU.mult)
            nc.vector.scalar_tensor_tensor(cbs[:], cbs[:], -1.0, bn2b[:], ALU.mult, ALU.add)
            c4n = W.tile([6, 1006], F32, tag="c4n")
            nc.vector.tensor_scalar(out=c4n[:], in0=c4[:], scalar1=cgs[:, 0:1],
                                    scalar2=cbs[:, 0:1], op0=ALU.mult, op1=ALU.add)
            seq7f = P.tile([7, 512], F32)
            nc.vector.memset(seq7f[:], 0.0)
            nc.vector.tensor_tensor(out=seq7f[0:6, 0:503], in0=c4n[:, 0:1006:2],
                                    in1=c4n[:, 1:1006:2], op=ALU.max)
            nc.sync.dma_start(out=seq7f[6:7, :], in_=ones512_d[:])
            if taps:
                nc.sync.dma_start(out=tap["t_seq"][:], in_=seq7f[:])
            seq7 = P.tile([7, 512], GRU_DT)
            nc.vector.tensor_copy(seq7[:], seq7f[:])

            att_ps.__exit__(None, None, None)
            PA_pool.__exit__(None, None, None)

            # =============== GRU ===============
            with (
                tc.tile_pool(name="gsb", bufs=2) as G,
                tc.tile_pool(name="gps", bufs=2, space="PSUM") as GP,
            ):
                hist = P.tile([128, 16 * ns], GRU_DT)
                prev_loc = h0loc
                h_outs = []
                for t in range(ns):
                    hrep = G.tile([128, 512], GRU_DT, tag="hrep")
                    if t == 0:
                        nc.sync.dma_start(out=hrep[:], in_=h0rep_d[:])
                    else:
                        hsb16 = G.tile([128, 16], GRU_DT, tag="hsb16")
                        nc.sync.dma_start(
                            out=hsb16[:],
                            in_=h_outs[-1][:].rearrange("(p k) -> p k", p=128))
                        nc.vector.tensor_copy(
                            hrep[:], hsb16[:].unsqueeze(2).broadcast_to((128, 16, 32)))
                        # hist for step t-1 = leaky(h_t-1)
                        _leaky(nc.vector, hist[:, t - 1 : t - 1 + 15 * ns + 1 : ns],
                               hsb16[:])
                    xrep = G.tile([7, 32], GRU_DT, tag="xrep")
                    nc.vector.tensor_copy(
                        xrep[:], seq7[:, t : t + 1].unsqueeze(2).broadcast_to((7, 1, 32)))
                    ps_g = GP.tile([128, 256], F32, tag="psg")
                    for g in range(4):
                        tp = (0, 32 * g)
                        nc.tensor.matmul(ps_g[32 * g : 32 * g + 32, :],
                                         lhsT=xrep[:, :],
                                         rhs=wtiny[:, 256 * g : 256 * g + 256],
                                         start=True, stop=False, tile_position=tp)
                        for k in range(16):
                            nc.tensor.matmul(
                                ps_g[32 * g : 32 * g + 32, 0:192],
                                lhsT=hrep[:, 32 * k : 32 * k + 32],
                                rhs=whrz[:, 768 * k + 192 * g : 768 * k + 192 * g + 192],
                                start=False, stop=(k == 15), tile_position=tp)
                    rz = G.tile([128, 128], F32, tag="rz")
                    nc.scalar.activation(rz[:], ps_g[:, 0:128], AF.Sigmoid)
                    nt = G.tile([128, 64], F32, tag="nt")
                    nc.vector.tensor_tensor(out=nt[:], in0=rz[:, 0:64],
                                            in1=ps_g[:, 128:192], op=ALU.mult)
                    nc.vector.tensor_tensor(out=nt[:], in0=nt[:],
                                            in1=ps_g[:, 192:256], op=ALU.add)
                    nc.scalar.activation(nt[:], nt[:], AF.Tanh)
                    hnew = G.tile([128, 64], F32, tag="hnew")
                    nc.vector.tensor_tensor(out=hnew[:], in0=prev_loc[:], in1=nt[:],
                                            op=ALU.subtract)
                    nc.vector.tensor_tensor(out=hnew[:], in0=hnew[:], in1=rz[:, 64:128],
                                            op=ALU.mult)
                    nc.vector.tensor_tensor(out=hnew[:], in0=hnew[:], in1=nt[:],
                                            op=ALU.add)
                    prev_loc = hnew
                    hnew16 = G.tile([128, 64], GRU_DT, tag="hnew16")
                    nc.vector.tensor_copy(hnew16[:], hnew[:])
                    h_in = DR2.tile([256], GRU_DT, tag="hin")
                    h_out = DR2.tile([2048], GRU_DT, tag="hout")
                    nc.sync.dma_start(out=h_in[:], in_=hnew16[0:128:32, :])
                    nc.gpsimd.collective_compute("AllGather", ALU.bypass,
                                                 replica_groups=rg,
                                                 ins=[h_in.opt()], outs=[h_out.opt()])
                    h_outs.append(h_out)
                    if len(h_outs) > 2:
                        h_outs.pop(0)
                # tail: hist for last step + full-precision hT AllGather
                hh16 = G.tile([128, 16], GRU_DT, tag="hh16")
                nc.sync.dma_start(out=hh16[:],
                                  in_=h_outs[-1][:].rearrange("(p k) -> p k", p=128))
                _leaky(nc.vector, hist[:, ns - 1 : ns - 1 + 15 * ns + 1 : ns], hh16[:])
                hf_in = DR.tile([256], F32)
                hf_out = DR.tile([2048], F32)
                nc.sync.dma_start(out=hf_in[:], in_=prev_loc[0:128:32, :])
                nc.gpsimd.collective_compute("AllGather", ALU.bypass,
                                             replica_groups=rg,
                                             ins=[hf_in.opt()], outs=[hf_out.opt()])
                nc.sync.dma_start(out=out_h[:], in_=hf_out[:])
                if taps:
                    hsbt = G.tile([128, 16], F32, tag="hsbt")
                    nc.sync.dma_start(out=hsbt[:],
                                      in_=hf_out[:].rearrange("(p k) -> p k", p=128))
                    nc.sync.dma_start(out=tap["t_hsb"][:], in_=hsbt[:])
                    nc.sync.dma_start(out=tap["t_hnew"][:], in_=prev_loc[:])

            # =============== MLP head ===============
            head_ps = tc.tile_pool(name="pshead", bufs=1, space="PSUM")
            psS = head_ps.__enter__()
            ps_x = psS.tile([NXS, LP], F32)
            for k in range(16):
                nc.tensor.matmul(ps_x[:, 0:ns],
                                 lhsT=l1w[:, NXS * k : NXS * k + NXS],
                                 rhs=hist[:, ns * k : ns * k + ns],
                                 start=(k == 0), stop=(k == 15))
            x50 = W.tile([NXS, LP], F32, tag="x50")
            nc.vector.tensor_scalar(out=x50[:, 0:ns], in0=ps_x[:, 0:ns],
                                    scalar1=l1b[:, 0:1], scalar2=None, op0=ALU.add)
            if ns < LP:
                nc.vector.memset(x50[:, ns:LP], 0.0)
            if taps:
                nc.sync.dma_start(out=tap["t_x50"][:], in_=x50[:])
            x_in = DR.tile([NXS * LP], F32)
            x_out = DR.tile([NX * LP], F32)
            nc.sync.dma_start(out=x_in[:], in_=x50[:])
            nc.gpsimd.collective_compute("AllGather", ALU.bypass, replica_groups=rg,
                                         ins=[x_in.opt()], outs=[x_out.opt()])
            xch = []
            for c in range(4):
                rows = 128 if c < 3 else NX - 384
                t_ = P.tile([128, LP], F32, tag=f"xch{c}")
                nc.sync.dma_start(out=t_[0:rows, :],
                                  in_=x_out[c * 128 * LP : (c * 128 + rows) * LP])
                xch.append((t_, rows))
            ps_st3 = psS.tile([1, 2], F32, tag="ps_small3", bufs=3)
            for i, (t_, rows) in enumerate(xch):
                xsum = W.tile([128, 1], F32, tag="xsum")
                nc.vector.tensor_reduce(xsum[0:rows, :], t_[0:rows, :], AX.X, ALU.add)
                xsqf = W.tile([128, LP], F32, tag="xsqf")
                xsq = W.tile([128, 1], F32, tag="xsq")
                nc.scalar.activation(xsqf[0:rows, :], t_[0:rows, :], AF.Square,
                                     accum_out=xsq[0:rows, :])
                x2 = W.tile([128, 2], F32, tag="x2")
                if rows < 128:
                    nc.vector.memset(x2[:], 0.0)
                nc.vector.tensor_copy(x2[0:rows, 0:1], xsum[0:rows, :])
                nc.vector.tensor_copy(x2[0:rows, 1:2], xsq[0:rows, :])
                nc.tensor.matmul(ps_st3[:, :], lhsT=onescol[:, :], rhs=x2[:, :],
                                 start=(i == 0), stop=(i == 3))
            NTOT = float(NX * LP)
            st3 = W.tile([1, 2], F32, tag="st3")
            nc.vector.tensor_scalar(out=st3[:], in0=ps_st3[:], scalar1=1.0 / NTOT,
                                    scalar2=None, op0=ALU.mult)
            v3 = W.tile([1, 1], F32, tag="v3")
            nc.vector.scalar_tensor_tensor(v3[:], st3[:, 0:1], -1.0, st3[:, 0:1],
                                           ALU.mult, ALU.mult)
            nc.vector.tensor_tensor(out=v3[:], in0=st3[:, 1:2], in1=v3[:], op=ALU.add)
            nc.vector.tensor_scalar(out=v3[:], in0=v3[:], scalar1=EPS, scalar2=None,
                                    op0=ALU.add)
            nc.scalar.activation(v3[:], v3[:], AF.Sqrt)
            r3 = W.tile([1, 1], F32, tag="r3")
            nc.vector.reciprocal(r3[:], v3[:])
            g3 = W.tile([1, 1], F32, tag="g3")
            nc.vector.tensor_tensor(out=g3[:], in0=r3[:], in1=consts[:, 4:5], op=ALU.mult)
            b3 = W.tile([1, 1], F32, tag="b3")
            nc.vector.tensor_tensor(out=b3[:], in0=st3[:, 0:1], in1=g3[:], op=ALU.mult)
            nc.vector.scalar_tensor_tensor(b3[:], b3[:], -1.0, consts[:, 5:6],
                                           ALU.mult, ALU.add)
            if taps:
                stt_ = W.tile([1, 4], F32, tag="stt_")
                nc.vector.tensor_copy(stt_[:, 0:2], st3[:])
                nc.vector.tensor_copy(stt_[:, 2:3], g3[:])
                nc.vector.tensor_copy(stt_[:, 3:4], b3[:])
                nc.sync.dma_start(out=tap["t_stats"][:], in_=stt_[:])
            gb3 = W.tile([1, 2], F32, tag="gb3")
            nc.vector.tensor_copy(gb3[:, 0:1], g3[:])
            nc.vector.tensor_copy(gb3[:, 1:2], b3[:])
            ps_bc3 = psS.tile([128, 2], F32, tag="ps_small3", bufs=3)
            nc.tensor.matmul(ps_bc3[:, :], lhsT=onesrow[:, :], rhs=gb3[:, :],
                             start=True, stop=True)
            g3b = W.tile([128, 2], F32, tag="g3b")
            nc.vector.tensor_copy(g3b[:], ps_bc3[:])
            ps_y = psS.tile([1, LP], F32, tag="ps_small3", bufs=3)
            for i, (t_, rows) in enumerate(xch):
                v_ = W.tile([128, LP], F32, tag="v_")
                va = W.tile([128, LP], F32, tag="va")
                if rows < 128:
                    nc.vector.memset(v_[:], 0.0)
                    nc.vector.memset(va[:], 0.0)
                nc.vector.tensor_scalar(out=v_[0:rows, :], in0=t_[0:rows, :],
                                        scalar1=g3b[0:rows, 0:1], scalar2=g3b[0:rows, 1:2],
                                        op0=ALU.mult, op1=ALU.add)
                _abs(nc.vector, va[0:rows, :], v_[0:rows, :])
                nc.tensor.matmul(ps_y[:, :], lhsT=l2w06[:, i : i + 1], rhs=v_[:, :],
                                 start=(i == 0), stop=False)
                nc.tensor.matmul(ps_y[:, :], lhsT=l2w04[:, i : i + 1], rhs=va[:, :],
                                 start=False, stop=(i == 3))
            u_sb = P.tile([1, 512], F32)
            nc.vector.memset(u_sb[:], 0.0)
            nc.vector.tensor_scalar(out=u_sb[:, 0:LP], in0=ps_y[:, :],
                                    scalar1=consts[:, 6:7], scalar2=None, op0=ALU.add)
            _leaky(nc.vector, u_sb[:, 0:LP], u_sb[:, 0:LP])
            nc.vector.tensor_copy(u_sb[:, LP : LP + 4], lastin[:])
            if taps:
                nc.sync.dma_start(out=tap["t_u"][:], in_=u_sb[:])
            ps_u = psS.tile([128, 4], F32, tag="ps_small3", bufs=3)
            for c in range(4):
                nc.tensor.transpose(ps_u[:, c : c + 1],
                                    u_sb[:, 128 * c : 128 * c + 128], eye1[:, :])
            ut = W.tile([128, 4], F32, tag="ut")
            nc.vector.tensor_copy(ut[:], ps_u[:])
            ps_v = psS.tile([1, 50], F32, tag="ps_small3", bufs=3)
            for c in range(4):
                nc.tensor.matmul(ps_v[:, :], lhsT=ut[:, c : c + 1],
                                 rhs=l3wt[:, 50 * c : 50 * c + 50],
                                 start=(c == 0), stop=(c == 3))
            vv = W.tile([1, 50], F32, tag="vv")
            nc.vector.tensor_tensor(out=vv[:], in0=ps_v[:], in1=l3b[:], op=ALU.add)
            _leaky(nc.vector, vv[:], vv[:])
            ps_vt = psS.tile([50, 1], F32, tag="ps_small3", bufs=3)
            nc.tensor.transpose(ps_vt[:, :], vv[:, :], eye1[:, :])
            vt = W.tile([50, 1], F32, tag="vt")
            nc.vector.tensor_copy(vt[:], ps_vt[:])
            ps_o = psS.tile([1, 4], F32, tag="ps_small3", bufs=3)
            nc.tensor.matmul(ps_o[:, :], lhsT=vt[:, :], rhs=l4wt[:, :],
                             start=True, stop=True)
            xout = W.tile([1, 4], F32, tag="xout")
            nc.vector.tensor_tensor(out=xout[:], in0=ps_o[:], in1=l4b[:], op=ALU.add)
            nc.sync.dma_start(out=out_x[:], in_=xout[:])
            head_ps.__exit__(None, None, None)

    nc.compile()
    return nc


# ==================== host-side prep ====================

def prep_inputs(inp):
    f = lambda x: np.ascontiguousarray(np.asarray(x, dtype=np.float32))
    W1, b1 = f(inp["W1_w"]), f(inp["W1_b"])
    V = f(inp["V_w"])[0]
    Vb = float(np.asarray(inp["V_b"]).reshape(-1)[0])
    vH = float(V[H])
    Vh = V[:H]
    enc = f(inp["enc_output"])[:, 0, :]
    hidden = f(inp["hidden"]).reshape(-1)
    W2, b2 = f(inp["W2_w"]), f(inp["W2_b"])
    Wi, Whh = f(inp["gru_Wi"]), f(inp["gru_Wh"])
    bi, bh = f(inp["gru_bi"]), f(inp["gru_bh"])
    lin1w, lin1b = f(inp["lin1_w"]), f(inp["lin1_b"])
    lin2w = f(inp["lin2_w"])[0]
    lin2b = float(np.asarray(inp["lin2_b"]).reshape(-1)[0])
    lin3w, lin3b = f(inp["lin3_w"]), f(inp["lin3_b"])
    lin4w, lin4b = f(inp["lin4_w"]), f(inp["lin4_b"])

    vW1 = 0.6 * (Vh @ W1)
    vc = 0.6 * float(Vh @ b1) + Vb
    v7 = np.concatenate([vW1, [vc]]).astype(np.float32).reshape(7, 1)
    w1t7 = np.concatenate([W1.T, b1[None, :]], axis=0).astype(np.float32)
    v1abs_t = (0.4 * Vh).reshape(16, 128).T.copy()

    consts = np.zeros((1, 16), np.float32)
    consts[0, 0] = float(np.asarray(inp["bn1_g"]).reshape(-1)[0])
    consts[0, 1] = float(np.asarray(inp["bn1_b"]).reshape(-1)[0])
    consts[0, 2] = 0.6 * vH
    consts[0, 3] = 0.4 * vH
    consts[0, 4] = float(np.asarray(inp["bn3_g"]).reshape(-1)[0])
    consts[0, 5] = float(np.asarray(inp["bn3_b"]).reshape(-1)[0])
    consts[0, 6] = lin2b

    def convw(w):
        kk = w.shape[2]
        out = np.zeros((6, 6 * kk), np.float32)
        for k in range(kk):
            out[:, 6 * k : 6 * k + 6] = w[:, :, k].T
        return out

    Wh_r, Wh_z, Wh_n = Whh[:H], Whh[H : 2 * H], Whh[2 * H :]
    Wi_r, Wi_z, Wi_n = Wi[:H], Wi[H : 2 * H], Wi[2 * H :]
    bi_r, bi_z, bi_n = bi[:H], bi[H : 2 * H], bi[2 * H :]
    bh_r, bh_z, bh_n = bh[:H], bh[H : 2 * H], bh[2 * H :]

    gdt = ml_dtypes.bfloat16 if GRU_BF16 else np.float32
    in_maps = []
    for c in range(NCORES):
        sl = slice(256 * c, 256 * c + 256)
        w2t = np.zeros((128, 16 * 256), np.float32)
        for k in range(16):
            w2t[:, 256 * k : 256 * k + 256] = W2[sl][:, k::16].T
        whrz = np.zeros((128, 16 * 768), np.float32)
        for k in range(16):
            for g in range(4):
                usl = slice(256 * c + 64 * g, 256 * c + 64 * g + 64)
                b0 = 768 * k + 192 * g
                whrz[:, b0 : b0 + 64] = Wh_r[usl][:, k::16].T
                whrz[:, b0 + 64 : b0 + 128] = Wh_z[usl][:, k::16].T
                whrz[:, b0 + 128 : b0 + 192] = Wh_n[usl][:, k::16].T
        wtiny = np.zeros((7, 1024), np.float32)
        for g in range(4):
            usl = slice(256 * c + 64 * g, 256 * c + 64 * g + 64)
            b0 = 256 * g
            wtiny[0:6, b0 : b0 + 64] = Wi_r[usl].T
            wtiny[6, b0 : b0 + 64] = bi_r[usl] + bh_r[usl]
            wtiny[0:6, b0 + 64 : b0 + 128] = Wi_z[usl].T
            wtiny[6, b0 + 64 : b0 + 128] = bi_z[usl] + bh_z[usl]
            wtiny[6, b0 + 128 : b0 + 192] = bh_n[usl]
            wtiny[0:6, b0 + 192 : b0 + 256] = Wi_n[usl].T
            wtiny[6, b0 + 192 : b0 + 256] = bi_n[usl]
        l1w = np.zeros((128, 16 * NXS), np.float32)
        rsl = slice(NXS * c, NXS * c + NXS)
        for k in range(16):
            l1w[:, NXS * k : NXS * k + NXS] = lin1w[rsl][:, k::16].T
        l2w06 = np.zeros((128, 4), np.float32)
        l2w04 = np.zeros((128, 4), np.float32)
        for cc in range(4):
            rows = 128 if cc < 3 else NX - 384
            l2w06[0:rows, cc] = 0.6 * lin2w[128 * cc : 128 * cc + rows]
            l2w04[0:rows, cc] = 0.4 * lin2w[128 * cc : 128 * cc + rows]
        l3wt = np.zeros((128, 4 * 50), np.float32)
        for cc in range(4):
            nn = 128 if cc < 3 else L3IN - 384
            l3wt[0:nn, 50 * cc : 50 * cc + 50] = lin3w[:, 128 * cc : 128 * cc + nn].T
        m = {
            "enc7": np.concatenate(
                [enc[sl].T, np.ones((1, 256), np.float32)], axis=0),
            "w1t7": w1t7, "v1abs": v1abs_t, "v7": v7,
            "w2t": w2t, "b2row": b2[sl].reshape(1, 256),
            "ones1": np.ones((1, 1), np.float32),
            "onescol": np.ones((128, 1), np.float32),
            "onesrow": np.ones((1, 128), np.float32),
            "consts": consts, "encctx": enc.T,
            "c1w": convw(f(inp["c1_w"])), "c1b": f(inp["c1_b"]).reshape(6, 1),
            "c2w": convw(f(inp["c2_w"])), "c2b": f(inp["c2_b"]).reshape(6, 1),
            "c3w": convw(f(inp["c3_w"])), "c3b": f(inp["c3_b"]).reshape(6, 1),
            "c4w": convw(f(inp["c4_w"])), "c4b": f(inp["c4_b"]).reshape(6, 1),
            "bn2g": f(inp["bn2_g"]).reshape(6, 1), "bn2b": f(inp["bn2_b"]).reshape(6, 1),
            "whrz": whrz.astype(gdt), "wtiny": wtiny.astype(gdt),
            "h0sb": hidden.reshape(128, 16),
            "h0rep": np.repeat(hidden.reshape(128, 16), 32, axis=1).astype(gdt),
            "h0loc": np.repeat(hidden[sl].reshape(4, 64), 32, axis=0),
            "l1w": l1w.astype(gdt), "l1b": lin1b[rsl].reshape(NXS, 1),
            "l2w06": l2w06, "l2w04": l2w04,
            "l3wt": l3wt, "l3b": lin3b.reshape(1, 50),
            "l4wt": lin4w.T, "l4b": lin4b.reshape(1, 4),
            "lastin": f(inp["last_input"]).reshape(1, 4),
            "ones512": np.ones((1, 512), np.float32),
            "eye": np.ones((1, 1), np.float32),
        }
        in_maps.append({k: np.ascontiguousarray(v) for k, v in m.items()})
    return in_maps


_CACHE = {}


def kernel(**inputs):
    if "nc" not in _CACHE:
        _CACHE["nc"] = build(n_steps=LP, taps=False)
    in_maps = prep_inputs(inputs)
    res = bass_utils.run_bass_kernel_spmd(_CACHE["nc"], in_maps,
                                          core_ids=list(range(NCORES)))
    x = res.results[0]["out_x"].reshape(4).astype(np.float32)
    hT = res.results[0]["out_h"].reshape(1, 1, 2048).astype(np.float32)
    return x, hT
